# revision 1
# baseline (speedup 1.0000x reference)
"""Distributed TRN2 Bass kernel for nn_ArgmaxISAModule (sparse argmax-attention stack).

Reference (per layer li, fp32):
    KX     = einsum('hqd,dn->hqn', K[li], X)
    scores = einsum('hqn,hqm->hnm', KX, KX)
    mask   = scores >= rowmax(scores) - 0.5
    w      = mask / max(rowsum(mask),1) * (|rowmax| > 0.5)
    attn   = X + sum_h V[li,h] @ (X @ w[h])
    X      = attn + W2[li] @ relu(W1[li] @ attn + b1[li]) + b2[li]

Distribution: token dim n sharded across 8 cores (NL=256 columns each).
Key structure per layer:
  - KX for layer li+1 is computed during layer li's FFN via the decomposition
    KX = Kt@attn + (K@W2)@ff1 + K@b2 (K@W2, K@b2 host-precomputed), so the
    AllGather of KX overlaps the FFN2/Yt phases.
  - mask via ACT Sign: s = sign(scores + (0.5 - rowmax)) in {-1,0,1}, count
    from accum_out (sum s = 2*count - N), and the big matmul uses s directly
    with a rank-1 correction:  sum_n mask*ys = sum_n s*(ys/2) + colsum(ys/2),
    the colsum added into each PSUM m-tile via a K=1 ones matmul.
  - big matmul in transposed orientation: partial^T[m,d] accumulated per
    128-row m-tile; ReduceScatter runs in 4 chunks overlapping production.
All matmuls in float32r (TF32-like, 1 cyc/row); full-model rel err ~6e-3.
"""
import numpy as np

import concourse.bacc as bacc
import concourse.mybir as mybir
import concourse.tile as tile
from concourse import masks
from concourse.bass_utils import run_bass_kernel_spmd

L, H, Q, D, N, DFF = 8, 4, 64, 512, 2048, 2048
CORES = 8
NL = N // CORES          # 256 local columns
P = 128
KD = D // P              # 4 k-tiles over d
KF = DFF // P            # 16 tiles over dff
MT = N // P              # 16 m-tiles
F32 = mybir.dt.float32
F32R = mybir.dt.float32r
BF16 = mybir.dt.bfloat16
AF = mybir.ActivationFunctionType
AL = mybir.AluOpType

# (head, local n-tile) pairs ordered so adjacent entries use different PE
# row groups (head parity) and can overlap in the array.
HT = [(0, 0), (1, 0), (0, 1), (1, 1), (2, 0), (3, 0), (2, 1), (3, 1)]
ACT_MASK_TILES = set()

_cache = {}


def build(L_EFF=L):
    nc = bacc.Bacc(num_devices=CORES)
    x_in = nc.declare_dram_parameter("x", [D, NL], F32R, isOutput=False)
    kt_in = nc.declare_dram_parameter("kt", [L, D, H * Q], F32R, isOutput=False)
    vt_in = nc.declare_dram_parameter("vt", [L, H, D, D], BF16, isOutput=False)
    w1t_in = nc.declare_dram_parameter("w1t", [L, D, DFF], BF16, isOutput=False)
    b1_in = nc.declare_dram_parameter("b1r", [L, P, KF], F32, isOutput=False)
    w2t_in = nc.declare_dram_parameter("w2t", [L, DFF, D], BF16, isOutput=False)
    b2_in = nc.declare_dram_parameter("b2r", [L, P, KD], F32, isOutput=False)
    kw2t_in = nc.declare_dram_parameter("kw2t", [L, DFF, H * Q], BF16, isOutput=False)
    kb2_in = nc.declare_dram_parameter("kb2r", [L, P, 2], F32, isOutput=False)
    out_ext = nc.declare_dram_parameter("out", [D, NL], F32R, isOutput=True)

    from contextlib import ExitStack
    with tile.TileContext(nc) as tc:
        with ExitStack() as stack:
            pool = lambda name, bufs, **kw: stack.enter_context(
                tc.tile_pool(name=name, bufs=bufs, **kw))
            pw = pool("pw", 8)            # sign tiles 8x[128,2048] f32r
            pkxf = pool("pkxf", 2)        # gathered KX
            pyt = pool("pyt", 8)          # yts
            pstg = pool("pstg", 3)        # big-mm staging
            px = pool("px", 8)            # x tiles
            pat = pool("pat", 4)          # attn tiles
            pff = pool("pff", 1)          # ff1
            pwta = pool("pwta", 6)        # kt/vt streams
            pwtb = pool("pwtb", 4)        # w1f/w2k/kw2 streams
            pst = pool("pst", 20)         # small stats
            pmisc = pool("pmisc", 1)      # ident/attn_t/ones
            pmisc2 = pool("pmisc2", 2)    # kxl / c_sb
            ps_s = pool("ps_s", 2, space="PSUM")   # score chunks [128,1024]
            ps_m = pool("ps_m", 2, space="PSUM")   # kx/yt/big/tr/ff
            ps_x = pool("ps_x", 2, space="PSUM")   # FFN2 accumulators + c_ps
            dpool = pool("dram", 2, space="DRAM")

            ident = pmisc.tile([P, P], F32, tag="ident")
            masks.make_identity(nc, ident[:])

            # layer-0 X tiles
            x_tiles = []
            for j in range(KD):
                xt = px.tile([P, NL], F32R, tag="x")
                nc.sync.dma_start(xt[:], x_in[P * j:P * (j + 1), :])
                x_tiles.append(xt)

            def load_kt(li):
                ts = []
                for k in range(KD):
                    t = pwta.tile([P, H * Q], F32R, tag="kt")
                    nc.sync.dma_start(t[:], kt_in[li, P * k:P * (k + 1), :])
                    ts.append(t)
                return ts

            def emit_ag(kxl):
                ag_in = dpool.tile([H * Q, NL], F32, tag="ag_in")
                ag_out = dpool.tile([N, NL], F32, tag="ag_out")
                nc.sync.dma_start(
                    ag_in[:].rearrange("(j p) n -> p j n", p=P),
                    kxl[:].bitcast(F32),
                )
                nc.gpsimd.collective_compute(
                    "AllGather", AL.bypass,
                    replica_groups=[list(range(CORES))],
                    ins=[ag_in[:]], outs=[ag_out[:]],
                )
                return ag_out

            # layer-0 KX + AG
            kt_t = load_kt(0)
            kxl = pmisc2.tile([P, 2, NL], F32R, tag="kxl")
            for j in range(2):
                kx_ps = ps_m.tile([P, NL], F32, tag="mmps")
                for k in range(KD):
                    nc.tensor.matmul(
                        kx_ps[:], kt_t[k][:, P * j:P * (j + 1)], x_tiles[k][:],
                        start=(k == 0), stop=(k == KD - 1),
                    )
                nc.scalar.copy(kxl[:, j, :], kx_ps[:])
            ag_out = emit_ag(kxl)

            for li in range(L_EFF):
                b1_sb = pst.tile([P, KF], F32, tag="b1")
                nc.sync.dma_start(b1_sb[:], b1_in[li])
                b2_sb = pst.tile([P, KD], F32, tag="b2")
                nc.sync.dma_start(b2_sb[:], b2_in[li])

                # gathered KX -> kxf[q + 64*(h%2), h//2, m]
                kxf = pkxf.tile([P, 2, N], F32R, tag="kxf")
                ag_v = ag_out[:].rearrange("(c hq) n -> c hq n", c=CORES)
                for h in range(H):
                    po = Q * (h % 2)
                    nc.gpsimd.dma_start(
                        kxf[po:po + Q, h // 2, :].rearrange("q (c n) -> q c n", c=CORES),
                        ag_v[:, Q * h:Q * (h + 1), :].rearrange("c q n -> q c n"),
                    )

                # bf16 shadow of X for the Yt matmuls (lhsT)
                xb_tiles = []
                for j in range(KD):
                    xb = px.tile([P, NL], BF16, tag="xb")
                    nc.vector.tensor_copy(xb[:], x_tiles[j][:].bitcast(F32))
                    xb_tiles.append(xb)

                # V^T streams for Yt
                vt_t = {}
                for h in range(H):
                    for k in range(KD):
                        t = pwta.tile([P, D], BF16, tag="vt")
                        nc.sync.dma_start(t[:], vt_in[li, h, P * k:P * (k + 1), :])
                        vt_t[(h, k)] = t

                # ---- scores + sign-mask + yts per (h, t); C accumulates colsums
                w_tiles = {}
                yts_tiles = {}
                for idx, (h, t_i) in enumerate(HT):
                    po = Q * (h % 2)
                    lhs = kxl[po:po + Q, h // 2, P * t_i:P * (t_i + 1)]
                    chunks = []
                    mxs = []
                    for c in range(2):
                        sc_ps = ps_s.tile([P, N // 2], F32, tag="scps")
                        for c2 in range(2):
                            nc.tensor.matmul(
                                sc_ps[:, 512 * c2:512 * (c2 + 1)], lhs,
                                kxf[po:po + Q, h // 2,
                                    1024 * c + 512 * c2:1024 * c + 512 * (c2 + 1)],
                                start=True, stop=True,
                            )
                        m = pst.tile([P, 1], F32, tag="mx")
                        nc.vector.reduce_max(m[:], sc_ps[:], axis=mybir.AxisListType.X)
                        chunks.append(sc_ps)
                        mxs.append(m)
                    rowmax = pst.tile([P, 1], F32, tag="rmax")
                    nc.vector.tensor_tensor(rowmax[:], mxs[0][:], mxs[1][:], op=AL.max)
                    # mask = (scores >= rowmax - 0.5) with per-row count.
                    # DVE path: is_ge with add-reduce accum. ACT path (offload):
                    # mask = is_finite(sqrt(scores + (0.5 - rowmax))) - sqrt of
                    # a negative is NaN, sqrt(0)=0 is finite, so >= is exact.
                    wt = pw.tile([P, N], BF16, tag="w")
                    cnt2 = pst.tile([P, 2], F32, tag="cnt2")
                    if idx in ACT_MASK_TILES:
                        thrn = pst.tile([P, 1], F32, tag="thrn")
                        nc.vector.tensor_scalar(thrn[:], rowmax[:], -1.0, 0.5,
                                                AL.mult, AL.add)
                        for c in range(2):
                            nc.scalar.activation(chunks[c][:], chunks[c][:],
                                                 AF.Sqrt, bias=thrn[:], scale=1.0)
                            nc.scalar.activation(wt[:, 1024 * c:1024 * (c + 1)],
                                                 chunks[c][:], AF.Is_finite,
                                                 accum_out=cnt2[:, c:c + 1])
                    else:
                        thr = pst.tile([P, 1], F32, tag="thr")
                        nc.vector.tensor_scalar_sub(thr[:], rowmax[:], 0.5)
                        for c in range(2):
                            nc.vector.tensor_scalar(
                                wt[:, 1024 * c:1024 * (c + 1)], chunks[c][:],
                                thr[:], 0.0, AL.is_ge, AL.add,
                                accum_out=cnt2[:, c:c + 1],
                            )
                    w_tiles[(h, t_i)] = wt
                    ssum = pst.tile([P, 1], F32, tag="ssum")
                    nc.vector.tensor_tensor(ssum[:], cnt2[:, 0:1], cnt2[:, 1:2], op=AL.add)
                    rcp2 = pst.tile([P, 1], F32, tag="rcp2")
                    nc.vector.reciprocal(rcp2[:], ssum[:])
                    actp = pst.tile([P, 1], F32, tag="actp")
                    nc.vector.tensor_single_scalar(actp[:], rowmax[:], 0.5, op=AL.is_gt)
                    actn = pst.tile([P, 1], F32, tag="actn")
                    nc.vector.tensor_single_scalar(actn[:], rowmax[:], -0.5, op=AL.is_lt)
                    act = pst.tile([P, 1], F32, tag="act")
                    nc.vector.tensor_tensor(act[:], actp[:], actn[:], op=AL.add)
                    rsc = pst.tile([P, 1], F32, tag="rsc")
                    nc.vector.tensor_tensor(rsc[:], rcp2[:], act[:], op=AL.mult)
                    yp = ps_m.tile([P, D], F32, tag="mmps")
                    for k in range(KD):
                        nc.tensor.matmul(
                            yp[:], xb_tiles[k][:, P * t_i:P * (t_i + 1)], vt_t[(h, k)][:],
                            start=(k == 0), stop=(k == KD - 1),
                        )
                    yts = pyt.tile([P, D], BF16, tag="yts")
                    nc.scalar.activation(yts[:], yp[:], AF.Identity, bias=0.0,
                                         scale=rsc[:])
                    yts_tiles[(h, t_i)] = yts

                # ---- big matmul + chunked ReduceScatter
                rs_in = dpool.tile([N, D], F32, tag="rs_in")
                rs_out = dpool.tile([NL, D], F32, tag="rs_out")
                attn_t = pmisc.tile([P, NL // P, D], F32, tag="attn_t")
                for mt in range(MT):
                    bp = ps_m.tile([P, D], F32, tag="mmps")
                    for i, ht in enumerate(HT):
                        nc.tensor.matmul(
                            bp[:], w_tiles[ht][:, P * mt:P * (mt + 1)], yts_tiles[ht][:],
                            start=(i == 0), stop=(i == len(HT) - 1),
                        )
                    stg = pstg.tile([P, D], F32, tag="bigstg")
                    nc.scalar.copy(stg[:], bp[:])
                    nc.sync.dma_start(rs_in[P * mt:P * (mt + 1), :], stg[:])
                nc.gpsimd.collective_compute(
                    "ReduceScatter", AL.add,
                    replica_groups=[list(range(CORES))],
                    ins=[rs_in[:]], outs=[rs_out[:]],
                )
                nc.sync.dma_start(
                    attn_t[:], rs_out[:].rearrange("(t p) d -> p t d", p=P)
                )

                # ---- transpose + residual: attn[d, m] = attn_out^T + X
                attn_sb = []
                for j in range(KD):
                    a = pat.tile([P, NL], F32R, tag="attn")
                    attn_sb.append(a)
                for t_i in range(NL // P):
                    for j in range(KD):
                        tp = ps_m.tile([P, P], F32, tag="mmps")
                        nc.tensor.transpose(
                            tp[:], attn_t[:, t_i, P * j:P * (j + 1)], ident[:]
                        )
                        nc.vector.scalar_tensor_tensor(
                            attn_sb[j][:, P * t_i:P * (t_i + 1)],
                            tp[:], 1.0, x_tiles[j][:, P * t_i:P * (t_i + 1)].bitcast(F32),
                            op0=AL.mult, op1=AL.add,
                        )

                # bf16 shadow of attn for FFN1
                ab_tiles = []
                for j in range(KD):
                    ab = pat.tile([P, NL], BF16, tag="ab")
                    nc.vector.tensor_copy(ab[:], attn_sb[j][:].bitcast(F32))
                    ab_tiles.append(ab)

                # ---- KX(li+1) part 1: Kt @ attn (runs while FFN1 streams)
                kx_ps_next = None
                if li < L_EFF - 1:
                    kt_t = load_kt(li + 1)
                    kx_ps_next = []
                    for j in range(2):
                        kx_ps = ps_m.tile([P, NL], F32, tag="mmps")
                        for k in range(KD):
                            nc.tensor.matmul(
                                kx_ps[:], kt_t[k][:, P * j:P * (j + 1)], attn_sb[k][:],
                                start=(k == 0), stop=False,
                            )
                        kx_ps_next.append(kx_ps)

                # ---- FFN1
                ff1 = pff.tile([P, KF, NL], BF16, tag="ff1")
                for f in range(KF):
                    w1f = pwtb.tile([P, KD, P], BF16, tag="w1f")
                    nc.sync.dma_start(
                        w1f[:],
                        w1t_in[li, :, P * f:P * (f + 1)].rearrange(
                            "(k p) f -> p k f", p=P),
                    )
                    fp = ps_x.tile([P, NL], F32, tag="xps")
                    for k in range(KD):
                        nc.tensor.matmul(
                            fp[:], w1f[:, k, :], ab_tiles[k][:],
                            start=(k == 0), stop=(k == KD - 1),
                        )
                    nc.scalar.activation(
                        ff1[:, f, :], fp[:], AF.Relu,
                        bias=b1_sb[:, f:f + 1], scale=1.0,
                    )

                # ---- KX(li+1) part 2: += (K@W2)@ff1, + K@b2, then AllGather
                if li < L_EFF - 1:
                    kb2_sb = pst.tile([P, 2], F32, tag="kb2")
                    nc.sync.dma_start(kb2_sb[:], kb2_in[li + 1])
                    kw2_t = []
                    for k in range(KF):
                        t = pwtb.tile([P, H * Q], BF16, tag="kw2")
                        nc.sync.dma_start(t[:], kw2t_in[li + 1, P * k:P * (k + 1), :])
                        kw2_t.append(t)
                    kxl = pmisc2.tile([P, 2, NL], F32R, tag="kxl")
                    for j in range(2):
                        kx_ps = kx_ps_next[j]
                        for k in range(KF):
                            nc.tensor.matmul(
                                kx_ps[:], kw2_t[k][:, P * j:P * (j + 1)], ff1[:, k, :],
                                start=False, stop=(k == KF - 1),
                            )
                        nc.scalar.activation(
                            kxl[:, j, :], kx_ps[:], AF.Identity,
                            bias=kb2_sb[:, j:j + 1], scale=1.0,
                        )
                    ag_out = emit_ag(kxl)

                # ---- FFN2 in two groups of two d-tiles
                new_x = []
                for g in range(2):
                    xps_list = []
                    for jj in range(2):
                        xps_t = ps_x.tile([P, NL], F32, tag="xps")
                        xps_list.append(xps_t)
                    for k in range(KF):
                        w2k = pwtb.tile([P, D], BF16, tag="w2k")
                        nc.sync.dma_start(w2k[:], w2t_in[li, P * k:P * (k + 1), :])
                        for jj in range(2):
                            j = 2 * g + jj
                            nc.tensor.matmul(
                                xps_list[jj][:], w2k[:, P * j:P * (j + 1)], ff1[:, k, :],
                                start=(k == 0), stop=(k == KF - 1),
                            )
                    for jj in range(2):
                        j = 2 * g + jj
                        xn = px.tile([P, NL], F32R, tag="x")
                        nc.vector.scalar_tensor_tensor(
                            xn[:], xps_list[jj][:], b2_sb[:, j:j + 1],
                            attn_sb[j][:].bitcast(F32),
                            op0=AL.add, op1=AL.add,
                        )
                        new_x.append(xn)
                x_tiles = new_x

            for j in range(KD):
                nc.sync.dma_start(out_ext[P * j:P * (j + 1), :], x_tiles[j][:])

    nc.finalize()
    return nc


def kernel(**inputs) -> np.ndarray:
    import ml_dtypes
    X = np.ascontiguousarray(inputs["X"], dtype=np.float32)
    K = np.asarray(inputs["K"], dtype=np.float32)
    V = np.asarray(inputs["V"], dtype=np.float32)
    W1 = np.asarray(inputs["W1"], dtype=np.float32)
    b1 = np.asarray(inputs["b1"], dtype=np.float32)
    W2 = np.asarray(inputs["W2"], dtype=np.float32)
    b2 = np.asarray(inputs["b2"], dtype=np.float32)

    kt = np.ascontiguousarray(K.reshape(L, H * Q, D).transpose(0, 2, 1))
    vt = np.ascontiguousarray(V.transpose(0, 1, 3, 2)).astype(ml_dtypes.bfloat16)
    w1t = np.ascontiguousarray(W1.transpose(0, 2, 1)).astype(ml_dtypes.bfloat16)
    w2t = np.ascontiguousarray(W2.transpose(0, 2, 1)).astype(ml_dtypes.bfloat16)
    b1r = np.ascontiguousarray(b1.reshape(L, KF, P).transpose(0, 2, 1))
    b2r = np.ascontiguousarray(b2.reshape(L, KD, P).transpose(0, 2, 1))

    # KX(li) = Kt(li)@attn + (K(li)@W2(li-1))@ff1 + K(li)@b2(li-1) for li>=1
    kr = K.reshape(L, H * Q, D).astype(np.float64)
    kw2t = np.zeros((L, DFF, H * Q), ml_dtypes.bfloat16)
    kb2 = np.zeros((L, H * Q, 1), np.float32)
    for li in range(1, L):
        kw2t[li] = (kr[li] @ W2[li - 1].astype(np.float64)).T.astype(ml_dtypes.bfloat16)
        kb2[li] = (kr[li] @ b2[li - 1].astype(np.float64)).astype(np.float32)
    kb2r = np.ascontiguousarray(kb2.reshape(L, 2, P).transpose(0, 2, 1))

    if "nc" not in _cache:
        _cache["nc"] = build()
    nc = _cache["nc"]

    in_maps = []
    for c in range(CORES):
        in_maps.append({
            "x": np.ascontiguousarray(X[:, c * NL:(c + 1) * NL]),
            "kt": kt, "vt": vt, "w1t": w1t, "b1r": b1r,
            "w2t": w2t, "b2r": b2r, "kw2t": kw2t, "kb2r": kb2r,
        })
    res = run_bass_kernel_spmd(nc, in_maps, core_ids=list(range(CORES)))
    out = np.concatenate([res.results[c]["out"] for c in range(CORES)], axis=1)
    return out.astype(np.float32)


if __name__ == "__main__":
    print("smoke build only")
    build()
    print("build ok")

